# revision 1
# baseline (speedup 1.0000x reference)
"""Trainium2 Bass kernel for NaiveFourierKANLayer — hybrid-shard v5.

y[b,j] = sum_{i,g} cos(x[b,i]*k_g) * W[0,j,i,g] + sin(x[b,i]*k_g) * W[1,j,i,g]

B=4096, I=128, O=512, G=300.  Sharding: 4 batch-shards x 2 g-shards.
Core m: batch rows [(m%4)*1024, +1024), g range [(m//4)*150, +150).
Host sums core pairs (m, m+4) and concatenates the 4 batch shards.

One pass, 8 PSUM banks, 1024-wide trig tiles.  Per g:
  t0 = (x*k') + MAGIC       (DVE ts-dual: rounds a=x*k' to int grid)
  n  = t0 - MAGIC           (DVE ts)
  f  = (x*k') - n           (DVE scalar_tensor_tensor, in [-0.5,0.5])
  sn = Sin(2pi*f)           (ACT fp16 out)
  even g: fc = wrap(f+.25) (DVE);  cs = Sin(2pi*fc)        (ACT)
  odd  g: fc = |f| (ACT);          cs = Sin(pi/2-2pi*fc)   (ACT)
  16 fp16 matmuls [K=128 i][M=128 b][N=512 j] accumulating 8 PSUM banks.

Head latency: x+k loads as three Sync DMAs (first quarter lands early),
g=0 trig runs in quarter/half pieces, g=0 W arrives as two half DMAs
(sin plane first) on GpSimd, W for g=1..4 prefetches via the idle ACT
sequencer, and 19 warm-up matmuls on a dummy tile burn the PE p-state
ramp while the inputs are still in flight.  Steady-state W DMAs issue
from GpSimd's SWDGE so Sync's sequencer never backlogs.  Output is one
[128, 8*512] fp16 tile, four DMAs alternating Sync/ACT queues as the
PSUM drain copies complete; host inverts the chunk-major layout and
sums g-shard pairs.

Measured: ~545-547us HW exec (8 cores), rel err ~3.1e-4 vs fp32 ref
(was 583-591us fp32r G-sharded baseline).  PE stream: 2400 fp16 matmuls
at 216ns median start-to-start (1 row/cycle @2.4GHz, p50=p90=p99), plus
~13us head (fixed ~5us NEFF entry barrier/ucode + input DMA + g0 trig)
and ~13us tail (drain+DMA ~5us + fixed ~7us NEFF semaphore-clear
epilogue).  fp8 DoubleRow measured only 2x MACs/cycle on HW (not the
cost model's 4x), so fp8 error-compensation cannot beat fp16 here.
"""
import numpy as np

B, I, O, G = 4096, 128, 512, 300
NCORES = 8
NB = 4                      # batch shards
NG = 2                      # g shards
BLOC = B // NB              # 1024
GLOC = G // NG              # 150
NCHUNK = BLOC // 128        # 8

MAGIC = float(np.float32(1.5 * 2 ** 23))
S2PI = float(np.float32(6.2831845))   # slightly < 2*pi so |f|*S2PI <= pi

_compiled = None


def _build():
    import concourse.bass as bass  # noqa: F401
    import concourse.mybir as mybir
    import concourse.tile as tile
    from concourse import bacc
    from concourse.alu_op_type import AluOpType

    f32 = mybir.dt.float32
    f16 = mybir.dt.float16
    Sin = mybir.ActivationFunctionType.Sin
    Abs = mybir.ActivationFunctionType.Abs

    nc = bacc.Bacc("TRN2", target_bir_lowering=False, debug=False,
                   num_devices=NCORES)
    # xk = [k' (GLOC cols) | x (BLOC cols)] in one input tensor, two DMAs
    XKW = GLOC + BLOC
    SPLIT = GLOC + BLOC // 2
    xk_d = nc.dram_tensor("xk", [I, XKW], f32, kind="ExternalInput").ap()
    w_d = nc.dram_tensor("w", [GLOC, I, 2 * O], f16, kind="ExternalInput").ap()
    y_d = nc.dram_tensor("yp", [I, NCHUNK * O], f16,
                         kind="ExternalOutput").ap()

    with tile.TileContext(nc) as tc:
        with (
            tc.tile_pool(name="inp", bufs=1) as inp,
            tc.tile_pool(name="wpool", bufs=4) as wpool,
            tc.tile_pool(name="trig", bufs=6) as trig,
            tc.tile_pool(name="psum", bufs=1, space="PSUM") as pp,
        ):
            xk = inp.tile([I, XKW], f32)
            Q = GLOC + BLOC // 4
            nc.sync.dma_start(xk[:, 0:Q], xk_d[:, 0:Q])
            nc.sync.dma_start(xk[:, Q:SPLIT], xk_d[:, Q:SPLIT])

            sv = xk[:, 0:GLOC]
            xt = xk[:, GLOC:XKW]
            bias_ph = inp.tile([I, 1], f32)
            nc.vector.memset(bias_ph[:], float(np.float32(np.pi / 2)))
            dum = inp.tile([I, O], f16)
            nc.vector.memset(dum[:], 0.0)
            o_all = inp.tile([I, NCHUNK * O], f16)

            ps = [pp.tile([128, O], f32, tag=f"ps{c}", name=f"ps{c}")
                  for c in range(NCHUNK)]
            # warm-up: burn the PE p-state ramp on dummies during the head
            for _ in range(19):
                nc.tensor.matmul(ps[0][:], dum[:, 0:128], dum[:],
                                 start=True, stop=True)

            H = BLOC // 2
            # prefetch W for g=0..4 before the loop: g0 halves on GpSimd
            # (sin plane first), g1-g4 on the idle ACT sequencer so the
            # early stream never waits on the single GpSimd SWDGE queue
            pre = {}
            for g in range(3):
                wt = wpool.tile([I, 2 * O], f16, tag="wt", name="wt")
                if g == 0:
                    # GpSimd queue order: sin plane (earliest matmuls),
                    # x upper half (sin p3 trig), cos plane (cos matmuls)
                    nc.gpsimd.dma_start(wt[:, 0:O], w_d[0, :, 0:O])
                    nc.gpsimd.dma_start(xk[:, SPLIT:XKW], xk_d[:, SPLIT:XKW])
                    nc.gpsimd.dma_start(wt[:, O:2 * O], w_d[0, :, O:2 * O])
                else:
                    nc.scalar.dma_start(wt[:], w_d[g])
                pre[g] = wt
            for g in range(GLOC):
                # [:, 0:O]=sin(d1) first, [:, O:2O]=cos(d0)
                if g < 3:
                    wt = pre[g]
                else:
                    wt = wpool.tile([I, 2 * O], f16, tag="wt", name="wt")
                    nc.gpsimd.dma_start(wt[:], w_d[g])
                ws = wt[:, 0:O]
                wc = wt[:, O:2 * O]

                t0 = trig.tile([I, BLOC], f32, tag="t0", name="t0")
                n = trig.tile([I, BLOC], f32, tag="n", name="n")
                f = trig.tile([I, BLOC], f32, tag="f", name="f")
                fc = trig.tile([I, BLOC], f32, tag="fc", name="fc")
                sn = trig.tile([I, BLOC], f16, tag="sn", name="sn")
                cs = trig.tile([I, BLOC], f16, tag="cs", name="cs")
                kg = sv[:, g:g + 1]
                # g=0 runs in pieces so the first matmuls start after only
                # a quarter of the x DMA + a 256-wide trig chain
                parts = ((0, H // 2), (H // 2, H), (H, BLOC)) if g == 0 \
                    else ((0, BLOC),)
                for lo, hi in parts:
                    s_ = slice(lo, hi)
                    nc.vector.tensor_scalar(t0[:, s_], xt[:, s_], kg, MAGIC,
                                            AluOpType.mult, AluOpType.add)
                    nc.vector.tensor_scalar(n[:, s_], t0[:, s_], MAGIC, None,
                                            AluOpType.subtract)
                    nc.vector.scalar_tensor_tensor(f[:, s_], xt[:, s_], kg,
                                                   n[:, s_], AluOpType.mult,
                                                   AluOpType.subtract)
                    nc.scalar.activation(sn[:, s_], f[:, s_], Sin, scale=S2PI)
                    if g % 2 == 0:
                        nc.vector.add_range_wrap(fc[:, s_], f[:, s_],
                                                 0.25, 0.5, 1.0)
                        nc.scalar.activation(cs[:, s_], fc[:, s_], Sin,
                                             scale=S2PI)
                    else:
                        nc.scalar.activation(fc[:, s_], f[:, s_], Abs)
                        nc.scalar.activation(cs[:, s_], fc[:, s_], Sin,
                                             scale=-S2PI, bias=bias_ph[:, 0:1])
                if g == GLOC - 1:
                    # interleave (sin c, cos c) so bank c's accumulation
                    # closes early and its drain+DMA overlaps the rest
                    for c in range(NCHUNK):
                        nc.tensor.matmul(ps[c][:],
                                         sn[:, c * 128:(c + 1) * 128],
                                         ws, start=False, stop=False)
                        nc.tensor.matmul(ps[c][:],
                                         cs[:, c * 128:(c + 1) * 128],
                                         wc, start=False, stop=True)
                else:
                    for c in range(NCHUNK):
                        nc.tensor.matmul(ps[c][:],
                                         sn[:, c * 128:(c + 1) * 128],
                                         ws, start=(g == 0), stop=False)
                    for c in range(NCHUNK):
                        nc.tensor.matmul(ps[c][:],
                                         cs[:, c * 128:(c + 1) * 128],
                                         wc, start=False, stop=False)
            oeng = [nc.vector.tensor_copy, nc.scalar.copy] * (NCHUNK // 2)
            odma = [nc.sync.dma_start, nc.scalar.dma_start,
                    nc.sync.dma_start, nc.scalar.dma_start]
            for c in range(NCHUNK):
                oeng[c](o_all[:, c * O:(c + 1) * O], ps[c][:])
                if c % 2 == 1:
                    sl = slice((c - 1) * O, (c + 1) * O)
                    odma[c // 2](y_d[:, sl], o_all[:, sl])

    nc.compile()
    return nc


def _prep(x, fouriercoeffs):
    xt = np.ascontiguousarray(x.T.astype(np.float32, copy=False))  # [I, B]
    # fouriercoeffs[d, j, i, g] -> wp[g, i, (sin, cos), j]
    wp = np.ascontiguousarray(
        fouriercoeffs[::-1].transpose(3, 2, 0, 1)).astype(np.float16)
    wp = wp.reshape(G, I, 2 * O)
    ks = (np.arange(1, G + 1, dtype=np.float64) / (2 * np.pi)).astype(
        np.float32)
    in_maps = []
    for m in range(NCORES):
        bsl = slice((m % NB) * BLOC, (m % NB + 1) * BLOC)
        gsl = slice((m // NB) * GLOC, (m // NB + 1) * GLOC)
        xkm = np.empty((I, GLOC + BLOC), dtype=np.float32)
        xkm[:, :GLOC] = ks[gsl]
        xkm[:, GLOC:] = xt[:, bsl]
        in_maps.append({
            "xk": xkm,
            "w": np.ascontiguousarray(wp[gsl]),
        })
    return in_maps


def kernel(x, fouriercoeffs):
    global _compiled
    from concourse.bass_utils import run_bass_kernel_spmd

    if _compiled is None:
        _compiled = _build()
    in_maps = _prep(np.asarray(x), np.asarray(fouriercoeffs))
    res = run_bass_kernel_spmd(_compiled, in_maps, core_ids=list(range(NCORES)))
    # yp is [128, 8*512] chunk-major: yp[p, c*512+j] = y[c*128+p, j]
    parts = []
    for m in range(NB):
        s = (res.results[m]["yp"].astype(np.float32)
             + res.results[m + NB]["yp"].astype(np.float32))
        parts.append(s.reshape(I, NCHUNK, O).transpose(1, 0, 2)
                     .reshape(BLOC, O))
    return np.concatenate(parts, axis=0).astype(np.float32)



# revision 3
# speedup vs baseline: 1.1298x; 1.1298x over previous
"""Trainium2 Bass kernel for NaiveFourierKANLayer — mixed fp16/fp8 v6.

y[b,j] = sum_{i,g} cos(x[b,i]*k_g) * W[0,j,i,g] + sin(x[b,i]*k_g) * W[1,j,i,g]

B=4096, I=128, O=512, G=300.  Sharding: 4 batch-shards x 2 g-shards.
Core m: batch rows [(m%4)*1024, +1024), g range [(m//4)*150, +150).
Host sums core pairs (m, m+4) and concatenates the 4 batch shards.

Mixed precision across g-planes: planes with g%3==2 (50 of 150 per
core) run as fp8e4 DoubleRow matmuls — one DR matmul per batch-chunk
computes sin+cos together (stationary = [sn|cs] pair [128,2,128],
moving = [Ws|Wc] pair [128,2,512]) in the same ~220ns a single fp16
matmul takes, i.e. 2x MACs/cycle (HW-verified; LDWEIGHTS fully hidden
for both dtypes, reload per MM is free).  Remaining planes stay fp16
(16 matmuls/g).  All W pre-scaled by 2^6 on host so fp8 weights sit in
e4m3's normal range (std 0.33); the 2^-6 is folded into the host-side
gather.  On-device f32->fp8 conversion is exact RTN (verified ==
ml_dtypes), so the measured error matches the numpy sim: rel err
~1.7e-2 vs the 2e-2 gate (all-fp8 would be 3.0e-2 — fails; all-fp16
is 3.1e-4 at 543us).

Per g:
  t0 = (x*k') + MAGIC       (DVE ts-dual: rounds a=x*k' to int grid)
  n  = t0 - MAGIC           (DVE ts)
  f  = (x*k') - n           (DVE scalar_tensor_tensor, in [-0.5,0.5])
  sn = Sin(2pi*f)           (ACT, fp16 out | fp8 out into pair slot 0)
  even g: fc = wrap(f+.25) (DVE);  cs = Sin(2pi*fc)        (ACT)
  odd  g: fc = |f| (ACT);          cs = Sin(pi/2-2pi*fc)   (ACT)
  fp16 g: 16 fp16 matmuls [K=128][M=128][N=512] over 8 PSUM banks
  fp8  g:  8 fp8 DR matmuls (sin+cos fused via slot pair) over 8 banks

Head latency: x+k loads as three Sync DMAs, g=0 trig in pieces, g=0 W
as two half DMAs on GpSimd, W for g=1,2 prefetched via the ACT
sequencer, 19 warm-up matmuls burn the PE p-state ramp.  Steady-state
W DMAs on GpSimd SWDGE.  Output: one [128, 8*512] fp16 tile, four
DMAs alternating Sync/ACT queues; host inverts chunk-major layout,
sums g-shard pairs, scales by 2^-6.

v5 (all-fp16) measured 543.4us; v6 projection ~460us.
"""
import numpy as np

B, I, O, G = 4096, 128, 512, 300
NCORES = 8
NB = 4                      # batch shards
NG = 2                      # g shards
BLOC = B // NB              # 1024
GLOC = G // NG              # 150
NCHUNK = BLOC // 128        # 8

MAGIC = float(np.float32(1.5 * 2 ** 23))
S2PI = float(np.float32(6.2831845))   # slightly < 2*pi so |f|*S2PI <= pi
WSCALE = 64.0               # W pre-scale: fp8 e4m3 normal range

IS8 = [g % 3 == 2 for g in range(GLOC)]   # fp8 plane pattern, alpha=1/3
N8 = sum(IS8)
N16 = GLOC - N8
# rank of g within its dtype class
_r8, _r16 = [], []
c8 = c16 = 0
for _g in range(GLOC):
    if IS8[_g]:
        _r8.append(c8); _r16.append(-1); c8 += 1
    else:
        _r16.append(c16); _r8.append(-1); c16 += 1

_compiled = None


def _build():
    import concourse.bass as bass  # noqa: F401
    import concourse.mybir as mybir
    import concourse.tile as tile
    from concourse import bacc
    from concourse.alu_op_type import AluOpType

    f32 = mybir.dt.float32
    f16 = mybir.dt.float16
    f8 = mybir.dt.float8e4
    Sin = mybir.ActivationFunctionType.Sin
    Abs = mybir.ActivationFunctionType.Abs
    DR = mybir.MatmulPerfMode.DoubleRow

    nc = bacc.Bacc("TRN2", target_bir_lowering=False, debug=False,
                   num_devices=NCORES)
    # xk = [k' (GLOC cols) | x (BLOC cols)] in one input tensor, two DMAs
    XKW = GLOC + BLOC
    SPLIT = GLOC + BLOC // 2
    xk_d = nc.dram_tensor("xk", [I, XKW], f32, kind="ExternalInput").ap()
    w_d = nc.dram_tensor("w", [N16, I, 2 * O], f16, kind="ExternalInput").ap()
    w8_d = nc.dram_tensor("w8", [N8, I, 2 * O], f8, kind="ExternalInput").ap()
    y_d = nc.dram_tensor("yp", [I, NCHUNK * O], f16,
                         kind="ExternalOutput").ap()

    with tile.TileContext(nc) as tc:
        with (
            tc.tile_pool(name="inp", bufs=1) as inp,
            tc.tile_pool(name="wpool", bufs=4) as wpool,
            tc.tile_pool(name="w8pool", bufs=4) as w8pool,
            tc.tile_pool(name="trig", bufs=6) as trig,
            tc.tile_pool(name="psum", bufs=1, space="PSUM") as pp,
        ):
            xk = inp.tile([I, XKW], f32)
            Q = GLOC + BLOC // 4
            nc.sync.dma_start(xk[:, 0:Q], xk_d[:, 0:Q])
            nc.sync.dma_start(xk[:, Q:SPLIT], xk_d[:, Q:SPLIT])

            sv = xk[:, 0:GLOC]
            xt = xk[:, GLOC:XKW]
            bias_ph = inp.tile([I, 1], f32)
            nc.vector.memset(bias_ph[:], float(np.float32(np.pi / 2)))
            dum = inp.tile([I, O], f16)
            nc.vector.memset(dum[:], 0.0)
            o_all = inp.tile([I, NCHUNK * O], f16)

            ps = [pp.tile([128, O], f32, tag=f"ps{c}", name=f"ps{c}")
                  for c in range(NCHUNK)]
            # warm-up: burn the PE p-state ramp on dummies during the head
            for _ in range(19):
                nc.tensor.matmul(ps[0][:], dum[:, 0:128], dum[:],
                                 start=True, stop=True)

            H = BLOC // 2
            # prefetch W for g=0..2 before the loop: g0 halves on GpSimd
            # (sin plane first), g1-g2 on the idle ACT sequencer so the
            # early stream never waits on the single GpSimd SWDGE queue
            pre = {}
            for g in range(3):
                if IS8[g]:
                    wt = w8pool.tile([I, 2, O], f8, tag="wt8", name="wt8")
                    nc.scalar.dma_start(wt[:], w8_d[_r8[g]])
                elif g == 0:
                    wt = wpool.tile([I, 2 * O], f16, tag="wt", name="wt")
                    # GpSimd queue order: sin plane (earliest matmuls),
                    # x upper half (sin p3 trig), cos plane (cos matmuls)
                    nc.gpsimd.dma_start(wt[:, 0:O], w_d[0, :, 0:O])
                    nc.gpsimd.dma_start(xk[:, SPLIT:XKW], xk_d[:, SPLIT:XKW])
                    nc.gpsimd.dma_start(wt[:, O:2 * O], w_d[0, :, O:2 * O])
                else:
                    wt = wpool.tile([I, 2 * O], f16, tag="wt", name="wt")
                    nc.scalar.dma_start(wt[:], w_d[_r16[g]])
                pre[g] = wt
            for g in range(GLOC):
                is8 = IS8[g]
                if g < 3:
                    wt = pre[g]
                elif is8:
                    wt = w8pool.tile([I, 2, O], f8, tag="wt8", name="wt8")
                    nc.gpsimd.dma_start(wt[:], w8_d[_r8[g]])
                else:
                    wt = wpool.tile([I, 2 * O], f16, tag="wt", name="wt")
                    nc.gpsimd.dma_start(wt[:], w_d[_r16[g]])

                t0 = trig.tile([I, BLOC], f32, tag="t0", name="t0")
                n = trig.tile([I, BLOC], f32, tag="n", name="n")
                f = trig.tile([I, BLOC], f32, tag="f", name="f")
                fc = trig.tile([I, BLOC], f32, tag="fc", name="fc")
                if is8:
                    pair = trig.tile([I, 2, BLOC], f8, tag="pair",
                                     name="pair")
                else:
                    sn = trig.tile([I, BLOC], f16, tag="sn", name="sn")
                    cs = trig.tile([I, BLOC], f16, tag="cs", name="cs")
                kg = sv[:, g:g + 1]
                # g=0 runs in pieces so the first matmuls start after only
                # a quarter of the x DMA + a 256-wide trig chain
                parts = ((0, H // 2), (H // 2, H), (H, BLOC)) if g == 0 \
                    else ((0, BLOC),)
                for lo, hi in parts:
                    s_ = slice(lo, hi)
                    sn_ap = pair[:, 0, s_] if is8 else sn[:, s_]
                    cs_ap = pair[:, 1, s_] if is8 else cs[:, s_]
                    nc.vector.tensor_scalar(t0[:, s_], xt[:, s_], kg, MAGIC,
                                            AluOpType.mult, AluOpType.add)
                    nc.vector.tensor_scalar(n[:, s_], t0[:, s_], MAGIC, None,
                                            AluOpType.subtract)
                    nc.vector.scalar_tensor_tensor(f[:, s_], xt[:, s_], kg,
                                                   n[:, s_], AluOpType.mult,
                                                   AluOpType.subtract)
                    nc.scalar.activation(sn_ap, f[:, s_], Sin, scale=S2PI)
                    if g % 2 == 0:
                        nc.vector.add_range_wrap(fc[:, s_], f[:, s_],
                                                 0.25, 0.5, 1.0)
                        nc.scalar.activation(cs_ap, fc[:, s_], Sin,
                                             scale=S2PI)
                    else:
                        nc.scalar.activation(fc[:, s_], f[:, s_], Abs)
                        nc.scalar.activation(cs_ap, fc[:, s_], Sin,
                                             scale=-S2PI, bias=bias_ph[:, 0:1])
                last = g == GLOC - 1
                if is8:
                    for c in range(NCHUNK):
                        nc.tensor.matmul(ps[c][:],
                                         pair[:, :, c * 128:(c + 1) * 128],
                                         wt[:], start=False, stop=last,
                                         perf_mode=DR)
                else:
                    ws = wt[:, 0:O]
                    wc = wt[:, O:2 * O]
                    if last:
                        # interleave (sin c, cos c) so bank c's accumulation
                        # closes early and its drain+DMA overlaps the rest
                        for c in range(NCHUNK):
                            nc.tensor.matmul(ps[c][:],
                                             sn[:, c * 128:(c + 1) * 128],
                                             ws, start=False, stop=False)
                            nc.tensor.matmul(ps[c][:],
                                             cs[:, c * 128:(c + 1) * 128],
                                             wc, start=False, stop=True)
                    else:
                        for c in range(NCHUNK):
                            nc.tensor.matmul(ps[c][:],
                                             sn[:, c * 128:(c + 1) * 128],
                                             ws, start=(g == 0), stop=False)
                        for c in range(NCHUNK):
                            nc.tensor.matmul(ps[c][:],
                                             cs[:, c * 128:(c + 1) * 128],
                                             wc, start=False, stop=False)
            oeng = [nc.vector.tensor_copy, nc.scalar.copy] * (NCHUNK // 2)
            odma = [nc.sync.dma_start, nc.scalar.dma_start,
                    nc.sync.dma_start, nc.scalar.dma_start]
            for c in range(NCHUNK):
                oeng[c](o_all[:, c * O:(c + 1) * O], ps[c][:])
                if c % 2 == 1:
                    sl = slice((c - 1) * O, (c + 1) * O)
                    odma[c // 2](y_d[:, sl], o_all[:, sl])

    nc.compile()
    return nc


def _prep(x, fouriercoeffs):
    from ml_dtypes import float8_e4m3

    xt = np.ascontiguousarray(x.T.astype(np.float32, copy=False))  # [I, B]
    # fouriercoeffs[d, j, i, g] -> wp[g, i, (sin, cos), j], scaled 2^6
    wp = np.ascontiguousarray(
        fouriercoeffs[::-1].transpose(3, 2, 0, 1)).astype(np.float32)
    wp = (wp * WSCALE).reshape(G, I, 2 * O)
    ks = (np.arange(1, G + 1, dtype=np.float64) / (2 * np.pi)).astype(
        np.float32)
    is8 = np.array(IS8, dtype=bool)
    in_maps = []
    for m in range(NCORES):
        bsl = slice((m % NB) * BLOC, (m % NB + 1) * BLOC)
        gsl = slice((m // NB) * GLOC, (m // NB + 1) * GLOC)
        xkm = np.empty((I, GLOC + BLOC), dtype=np.float32)
        xkm[:, :GLOC] = ks[gsl]
        xkm[:, GLOC:] = xt[:, bsl]
        wg = wp[gsl]
        in_maps.append({
            "xk": xkm,
            "w": np.ascontiguousarray(wg[~is8]).astype(np.float16),
            "w8": np.ascontiguousarray(wg[is8]).astype(float8_e4m3),
        })
    return in_maps


def kernel(x, fouriercoeffs):
    global _compiled
    from concourse.bass_utils import run_bass_kernel_spmd

    if _compiled is None:
        _compiled = _build()
    in_maps = _prep(np.asarray(x), np.asarray(fouriercoeffs))
    res = run_bass_kernel_spmd(_compiled, in_maps, core_ids=list(range(NCORES)))
    # yp is [128, 8*512] chunk-major: yp[p, c*512+j] = y[c*128+p, j]
    parts = []
    for m in range(NB):
        s = (res.results[m]["yp"].astype(np.float32)
             + res.results[m + NB]["yp"].astype(np.float32))
        parts.append((s * (1.0 / WSCALE)).reshape(I, NCHUNK, O)
                     .transpose(1, 0, 2).reshape(BLOC, O))
    return np.concatenate(parts, axis=0).astype(np.float32)


# revision 4
# speedup vs baseline: 1.1304x; 1.0005x over previous
"""Trainium2 Bass kernel for NaiveFourierKANLayer — mixed fp16/fp8 v6.

y[b,j] = sum_{i,g} cos(x[b,i]*k_g) * W[0,j,i,g] + sin(x[b,i]*k_g) * W[1,j,i,g]

B=4096, I=128, O=512, G=300.  Sharding: 4 batch-shards x 2 g-shards.
Core m: batch rows [(m%4)*1024, +1024), g range [(m//4)*150, +150).
Host sums core pairs (m, m+4) and concatenates the 4 batch shards.

Mixed precision across g-planes: planes with g%3==2 (50 of 150 per
core) run as fp8e4 DoubleRow matmuls — one DR matmul per batch-chunk
computes sin+cos together (stationary = [sn|cs] pair [128,2,128],
moving = [Ws|Wc] pair [128,2,512]) in the same ~220ns a single fp16
matmul takes, i.e. 2x MACs/cycle (HW-verified; LDWEIGHTS fully hidden
for both dtypes, reload per MM is free).  Remaining planes stay fp16
(16 matmuls/g).  All W pre-scaled by 2^6 on host so fp8 weights sit in
e4m3's normal range (std 0.33); the 2^-6 is folded into the host-side
gather.  On-device f32->fp8 conversion is exact RTN (verified ==
ml_dtypes), so the measured error matches the numpy sim: rel err
~1.7e-2 vs the 2e-2 gate (all-fp8 would be 3.0e-2 — fails; all-fp16
is 3.1e-4 at 543us).

Per g:
  t0 = (x*k') + MAGIC       (DVE ts-dual: rounds a=x*k' to int grid)
  n  = t0 - MAGIC           (DVE ts)
  f  = (x*k') - n           (DVE scalar_tensor_tensor, in [-0.5,0.5])
  sn = Sin(2pi*f)           (ACT, fp16 out | fp8 out into pair slot 0)
  even g: fc = wrap(f+.25) (DVE);  cs = Sin(2pi*fc)        (ACT)
  odd  g: fc = |f| (ACT);          cs = Sin(pi/2-2pi*fc)   (ACT)
  fp16 g: 16 fp16 matmuls [K=128][M=128][N=512] over 8 PSUM banks
  fp8  g:  8 fp8 DR matmuls (sin+cos fused via slot pair) over 8 banks

Head latency: x+k loads as three Sync DMAs, g=0 trig in pieces, g=0 W
as two half DMAs on GpSimd, W for g=1,2 prefetched via the ACT
sequencer, 19 warm-up matmuls burn the PE p-state ramp.  Steady-state
W DMAs on GpSimd SWDGE.  Output: one [128, 8*512] fp16 tile, four
DMAs alternating Sync/ACT queues; host inverts chunk-major layout,
sums g-shard pairs, scales by 2^-6.

v5 (all-fp16) measured 543.4us; v6 projection ~460us.
"""
import numpy as np

B, I, O, G = 4096, 128, 512, 300
NCORES = 8
NB = 4                      # batch shards
NG = 2                      # g shards
BLOC = B // NB              # 1024
GLOC = G // NG              # 150
NCHUNK = BLOC // 128        # 8

MAGIC = float(np.float32(1.5 * 2 ** 23))
S2PI = float(np.float32(6.2831845))   # slightly < 2*pi so |f|*S2PI <= pi
WSCALE = 64.0               # W pre-scale: fp8 e4m3 normal range

IS8 = [g % 3 == 2 for g in range(GLOC)]   # fp8 plane pattern, alpha=1/3
N8 = sum(IS8)
N16 = GLOC - N8
# rank of g within its dtype class
_r8, _r16 = [], []
c8 = c16 = 0
for _g in range(GLOC):
    if IS8[_g]:
        _r8.append(c8); _r16.append(-1); c8 += 1
    else:
        _r16.append(c16); _r8.append(-1); c16 += 1

_compiled = None


def _build():
    import concourse.bass as bass  # noqa: F401
    import concourse.mybir as mybir
    import concourse.tile as tile
    from concourse import bacc
    from concourse.alu_op_type import AluOpType

    f32 = mybir.dt.float32
    f16 = mybir.dt.float16
    f8 = mybir.dt.float8e4
    Sin = mybir.ActivationFunctionType.Sin
    Abs = mybir.ActivationFunctionType.Abs
    DR = mybir.MatmulPerfMode.DoubleRow

    nc = bacc.Bacc("TRN2", target_bir_lowering=False, debug=False,
                   num_devices=NCORES)
    # xk = [k' (GLOC cols) | x (BLOC cols)] in one input tensor, two DMAs
    XKW = GLOC + BLOC
    SPLIT = GLOC + BLOC // 2
    xk_d = nc.dram_tensor("xk", [I, XKW], f32, kind="ExternalInput").ap()
    w_d = nc.dram_tensor("w", [N16, I, 2 * O], f16, kind="ExternalInput").ap()
    w8_d = nc.dram_tensor("w8", [N8, I, 2 * O], f8, kind="ExternalInput").ap()
    y_d = nc.dram_tensor("yp", [I, NCHUNK * O], f16,
                         kind="ExternalOutput").ap()

    with tile.TileContext(nc) as tc:
        with (
            tc.tile_pool(name="inp", bufs=1) as inp,
            tc.tile_pool(name="wpool", bufs=4) as wpool,
            tc.tile_pool(name="w8pool", bufs=4) as w8pool,
            tc.tile_pool(name="trig", bufs=6) as trig,
            tc.tile_pool(name="psum", bufs=1, space="PSUM") as pp,
        ):
            xk = inp.tile([I, XKW], f32)
            Q = GLOC + BLOC // 4
            nc.sync.dma_start(xk[:, 0:Q], xk_d[:, 0:Q])
            nc.sync.dma_start(xk[:, Q:SPLIT], xk_d[:, Q:SPLIT])

            sv = xk[:, 0:GLOC]
            xt = xk[:, GLOC:XKW]
            bias_ph = inp.tile([I, 1], f32)
            nc.vector.memset(bias_ph[:], float(np.float32(np.pi / 2)))
            dum = inp.tile([I, O], f16)
            nc.vector.memset(dum[:], 0.0)
            o_all = inp.tile([I, NCHUNK * O], f16)

            ps = [pp.tile([128, O], f32, tag=f"ps{c}", name=f"ps{c}")
                  for c in range(NCHUNK)]
            # warm-up: burn the PE p-state ramp on dummies during the head
            for _ in range(19):
                nc.tensor.matmul(ps[0][:], dum[:, 0:128], dum[:],
                                 start=True, stop=True)

            H = BLOC // 2
            # prefetch W for g=0..2 before the loop: g0 halves on GpSimd
            # (sin plane first), g1-g2 on the idle ACT sequencer so the
            # early stream never waits on the single GpSimd SWDGE queue
            pre = {}
            for g in range(3):
                if IS8[g]:
                    wt = w8pool.tile([I, 2, O], f8, tag="wt8", name="wt8")
                    nc.scalar.dma_start(wt[:], w8_d[_r8[g]])
                elif g == 0:
                    wt = wpool.tile([I, 2 * O], f16, tag="wt", name="wt")
                    # GpSimd queue order: sin plane (earliest matmuls),
                    # x upper half (sin p3 trig), cos plane (cos matmuls)
                    nc.gpsimd.dma_start(wt[:, 0:O], w_d[0, :, 0:O])
                    nc.gpsimd.dma_start(xk[:, SPLIT:XKW], xk_d[:, SPLIT:XKW])
                    nc.gpsimd.dma_start(wt[:, O:2 * O], w_d[0, :, O:2 * O])
                else:
                    wt = wpool.tile([I, 2 * O], f16, tag="wt", name="wt")
                    nc.scalar.dma_start(wt[:], w_d[_r16[g]])
                pre[g] = wt
            for g in range(GLOC):
                is8 = IS8[g]
                if g < 3:
                    wt = pre[g]
                elif is8:
                    wt = w8pool.tile([I, 2, O], f8, tag="wt8", name="wt8")
                    nc.gpsimd.dma_start(wt[:], w8_d[_r8[g]])
                else:
                    wt = wpool.tile([I, 2 * O], f16, tag="wt", name="wt")
                    nc.gpsimd.dma_start(wt[:], w_d[_r16[g]])

                t0 = trig.tile([I, BLOC], f32, tag="t0", name="t0")
                n = trig.tile([I, BLOC], f32, tag="n", name="n")
                f = trig.tile([I, BLOC], f32, tag="f", name="f")
                fc = trig.tile([I, BLOC], f32, tag="fc", name="fc")
                if is8:
                    pair = trig.tile([I, 2, BLOC], f8, tag="pair",
                                     name="pair")
                else:
                    sn = trig.tile([I, BLOC], f16, tag="sn", name="sn")
                    cs = trig.tile([I, BLOC], f16, tag="cs", name="cs")
                kg = sv[:, g:g + 1]
                # g=0 runs in pieces so the first matmuls start after only
                # a quarter of the x DMA + a 256-wide trig chain
                parts = ((0, H // 2), (H // 2, H), (H, BLOC)) if g == 0 \
                    else ((0, BLOC),)
                for lo, hi in parts:
                    s_ = slice(lo, hi)
                    sn_ap = pair[:, 0, s_] if is8 else sn[:, s_]
                    cs_ap = pair[:, 1, s_] if is8 else cs[:, s_]
                    nc.vector.tensor_scalar(t0[:, s_], xt[:, s_], kg, MAGIC,
                                            AluOpType.mult, AluOpType.add)
                    nc.vector.tensor_scalar(n[:, s_], t0[:, s_], MAGIC, None,
                                            AluOpType.subtract)
                    nc.vector.scalar_tensor_tensor(f[:, s_], xt[:, s_], kg,
                                                   n[:, s_], AluOpType.mult,
                                                   AluOpType.subtract)
                    nc.scalar.activation(sn_ap, f[:, s_], Sin, scale=S2PI)
                    if g % 2 == 0:
                        nc.vector.add_range_wrap(fc[:, s_], f[:, s_],
                                                 0.25, 0.5, 1.0)
                        nc.scalar.activation(cs_ap, fc[:, s_], Sin,
                                             scale=S2PI)
                    else:
                        nc.scalar.activation(fc[:, s_], f[:, s_], Abs)
                        nc.scalar.activation(cs_ap, fc[:, s_], Sin,
                                             scale=-S2PI, bias=bias_ph[:, 0:1])
                last = g == GLOC - 1
                if is8:
                    for c in range(NCHUNK):
                        nc.tensor.matmul(ps[c][:],
                                         pair[:, :, c * 128:(c + 1) * 128],
                                         wt[:], start=False, stop=last,
                                         perf_mode=DR)
                else:
                    ws = wt[:, 0:O]
                    wc = wt[:, O:2 * O]
                    if last:
                        # interleave (sin c, cos c) so bank c's accumulation
                        # closes early and its drain+DMA overlaps the rest
                        for c in range(NCHUNK):
                            nc.tensor.matmul(ps[c][:],
                                             sn[:, c * 128:(c + 1) * 128],
                                             ws, start=False, stop=False)
                            nc.tensor.matmul(ps[c][:],
                                             cs[:, c * 128:(c + 1) * 128],
                                             wc, start=False, stop=True)
                    else:
                        for c in range(NCHUNK):
                            nc.tensor.matmul(ps[c][:],
                                             sn[:, c * 128:(c + 1) * 128],
                                             ws, start=(g == 0), stop=False)
                        for c in range(NCHUNK):
                            nc.tensor.matmul(ps[c][:],
                                             cs[:, c * 128:(c + 1) * 128],
                                             wc, start=False, stop=False)
            oeng = [nc.vector.tensor_copy, nc.scalar.copy] * (NCHUNK // 2)
            odma = [nc.sync.dma_start, nc.scalar.dma_start] * (NCHUNK // 2)
            for c in range(NCHUNK):
                if c == NCHUNK - 1:
                    # split the last bank's drain across both engines so
                    # the final (single-bank) DMA can start ~0.4us sooner
                    h = O // 2
                    nc.vector.tensor_copy(o_all[:, c * O:c * O + h],
                                          ps[c][:, 0:h])
                    nc.scalar.copy(o_all[:, c * O + h:(c + 1) * O],
                                   ps[c][:, h:O])
                else:
                    oeng[c](o_all[:, c * O:(c + 1) * O], ps[c][:])
                sl = slice(c * O, (c + 1) * O)
                odma[c](y_d[:, sl], o_all[:, sl])

    nc.compile()
    return nc


def _prep(x, fouriercoeffs):
    from ml_dtypes import float8_e4m3

    xt = np.ascontiguousarray(x.T.astype(np.float32, copy=False))  # [I, B]
    # fouriercoeffs[d, j, i, g] -> wp[g, i, (sin, cos), j], scaled 2^6
    wp = np.ascontiguousarray(
        fouriercoeffs[::-1].transpose(3, 2, 0, 1)).astype(np.float32)
    wp = (wp * WSCALE).reshape(G, I, 2 * O)
    ks = (np.arange(1, G + 1, dtype=np.float64) / (2 * np.pi)).astype(
        np.float32)
    is8 = np.array(IS8, dtype=bool)
    in_maps = []
    for m in range(NCORES):
        bsl = slice((m % NB) * BLOC, (m % NB + 1) * BLOC)
        gsl = slice((m // NB) * GLOC, (m // NB + 1) * GLOC)
        xkm = np.empty((I, GLOC + BLOC), dtype=np.float32)
        xkm[:, :GLOC] = ks[gsl]
        xkm[:, GLOC:] = xt[:, bsl]
        wg = wp[gsl]
        in_maps.append({
            "xk": xkm,
            "w": np.ascontiguousarray(wg[~is8]).astype(np.float16),
            "w8": np.ascontiguousarray(wg[is8]).astype(float8_e4m3),
        })
    return in_maps


def kernel(x, fouriercoeffs):
    global _compiled
    from concourse.bass_utils import run_bass_kernel_spmd

    if _compiled is None:
        _compiled = _build()
    in_maps = _prep(np.asarray(x), np.asarray(fouriercoeffs))
    res = run_bass_kernel_spmd(_compiled, in_maps, core_ids=list(range(NCORES)))
    # yp is [128, 8*512] chunk-major: yp[p, c*512+j] = y[c*128+p, j]
    parts = []
    for m in range(NB):
        s = (res.results[m]["yp"].astype(np.float32)
             + res.results[m + NB]["yp"].astype(np.float32))
        parts.append((s * (1.0 / WSCALE)).reshape(I, NCHUNK, O)
                     .transpose(1, 0, 2).reshape(BLOC, O))
    return np.concatenate(parts, axis=0).astype(np.float32)
